# revision 3
# baseline (speedup 1.0000x reference)
"""Trainium2 Bass kernel for Llama4TextExperts-style grouped MoE FFN (SwiGLU).

Full-input contract: kernel(**inputs) takes the complete unsharded tensors and
returns the full [4096, 1024] output. Internally: expert-parallel across the 8
NeuronCores — core e gets expert e's three weight matrices and that expert's
512-token group (tokens arrive pre-sorted by expert with equal group sizes).
All routing / transposition is done host-side in numpy; no collectives needed.

Per-core device program (three GEMMs, ~6.4 GFLOP):
  phase 1: gate^T = Wg^T-stationary @ x^T, up^T likewise; SwiGLU fused on
           ACT (Silu) + DVE (mul) into h^T [I, T] bf16 resident in SBUF.
  phase 2: y = h @ Wd with h^T slices stationary, Wd streaming from its
           natural [I, H] DRAM layout; y lands untransposed in PSUM.

Startup schedule (the PE stream is already at the bf16 streaming roofline,
so the win is at the edges):
  - 8 warm-up matmuls on zeroed SBUF run while the first DMAs land, so the
    HAM clock gate releases (1.2->2.4 GHz) right as real work starts.
  - it0/it1 use narrow i-major weight tiles ([P, 8, 128], one DMA each) so
    the first gate chain only waits for ~0.5 MB; x streams concurrently on
    the scalar HWDGE queue while weights use the sync HWDGE queue.
  - it2..15 use k-major bulk tiles (contiguous 1.5-2 KB DMA descriptor rows
    straight from the [H, I] weight layout - much cheaper to issue than
    column-chunked loads). Each DMA is gated on an earlier completion so at
    most ~3 are in flight and arrival order tracks consumption order.
  - the final down-proj token tile is split into shrinking chains
    (512/256/128/128 cols) so the exposed copy+DMA after the very last
    matmul is only [128, 128], written out on the otherwise-idle scalar
    queue.
"""

import numpy as np
import ml_dtypes

import concourse.bass as bass
import concourse.mybir as mybir
import concourse.tile as tile
from concourse.tile import add_dep_helper
from concourse import bacc
from concourse.bass_utils import run_bass_kernel_spmd

# Problem shape (hardcoded per contract)
E = 8          # experts == cores
T = 512        # tokens per expert group
H = 1024       # hidden
I = 2048       # intermediate
P = 128        # partitions
KT = H // P    # 8  k-tiles over hidden
IT = I // P    # 16 i-tiles over intermediate
MT = T // P    # 4  token tiles

BF16 = mybir.dt.bfloat16
F32 = mybir.dt.float32

_compiled = None


def _build():
    nc = bacc.Bacc(None)
    xT_d = nc.declare_dram_parameter("xT", [H, T], BF16, isOutput=False)
    wg_d = nc.declare_dram_parameter("wg", [H, I], BF16, isOutput=False)
    wu_d = nc.declare_dram_parameter("wu", [H, I], BF16, isOutput=False)
    wd_d = nc.declare_dram_parameter("wd", [I, H], BF16, isOutput=False)
    y_d = nc.declare_dram_parameter("y", [T, H], F32, isOutput=True)

    xT_r = xT_d.rearrange("(ko p) t -> p ko t", p=P)     # [128, 8, 512]
    wg_r = wg_d.rearrange("(ko p) i -> p ko i", p=P)     # [128, 8, 2048]
    wu_r = wu_d.rearrange("(ko p) i -> p ko i", p=P)
    wd_r = wd_d.rearrange("(io p) h -> p io h", p=P)     # [128, 16, 1024]

    with tile.TileContext(nc) as tc:
        with (
            tc.tile_pool(name="xpool", bufs=1) as xpool,
            tc.tile_pool(name="kpool", bufs=1) as kpool,
            tc.tile_pool(name="wdpool", bufs=1) as wdpool,
            tc.tile_pool(name="hpool", bufs=1) as hpool,
            tc.tile_pool(name="spool", bufs=3) as spool,
            tc.tile_pool(name="psum", bufs=2, space="PSUM") as psum,
        ):
            # PE warm-up: the HAM clock gate holds the PE at 1.2GHz until it
            # has been ~3.4us busy. 8 cold N=512 matmuls on zeroed scratch
            # cover the DMA-wait window so the real stream starts at 2.4GHz.
            warm = xpool.tile([P, 512], BF16, tag="warm", name="warm")
            nc.gpsimd.memset(warm[:], 0.0)
            pwarm = psum.tile([P, 512], F32, tag="pg", name="pwarm")
            for _ in range(8):
                nc.tensor.matmul(
                    pwarm[:], warm[:, 0:128], warm[:], start=True, stop=True
                )

            # --- x: four [P, 2, T] chunks on the scalar HWDGE queue, two in
            # flight (so x shares the pipe with the head weight loads without
            # flooding it).
            x_sb = xpool.tile([P, KT, T], BF16, tag="x", name="x_sb")
            dx = []
            for kq in range(4):
                call = nc.scalar.dma_start(
                    x_sb[:, 2 * kq:2 * kq + 2, :], xT_r[:, 2 * kq:2 * kq + 2, :]
                )
                if kq >= 2:
                    add_dep_helper(
                        call.ins, dx[kq - 2].ins, reason="x staging throttle"
                    )
                dx.append(call)

            # --- weights on the sync HWDGE queue, pipelined ~3 deep.
            # Head: narrow i-major tiles for it0/it1 (one small DMA each).
            d_chain = []  # completion chain for gating

            def sync_dma(dst, src, gate_idx):
                call = nc.sync.dma_start(dst, src)
                if gate_idx >= 0:
                    add_dep_helper(
                        call.ins, d_chain[gate_idx].ins,
                        reason="weight staging throttle",
                    )
                d_chain.append(call)
                return call

            wg_it0 = xpool.tile([P, KT, 128], BF16, tag="wg_it0", name="wg_it0")
            sync_dma(wg_it0[:], wg_r[:, :, 0:128], -1)
            wu_it0 = xpool.tile([P, KT, 128], BF16, tag="wu_it0", name="wu_it0")
            sync_dma(wu_it0[:], wu_r[:, :, 0:128], -1)
            wg_it1 = xpool.tile([P, KT, 128], BF16, tag="wg_it1", name="wg_it1")
            sync_dma(wg_it1[:], wg_r[:, :, 128:256], -1)
            wu_it1 = xpool.tile([P, KT, 128], BF16, tag="wu_it1", name="wu_it1")
            sync_dma(wu_it1[:], wu_r[:, :, 128:256], 0)

            # Bulk: k-major tiles straight from the [H, I] layout. Rows are
            # contiguous 1.5KB/2KB descriptors. b1 covers i-cols 256:1024
            # (it2..7), b2 covers 1024:2048 (it8..15). wg/wu interleaved per
            # k-tile so the gate and up chains starve equally late.
            wg_b1, wu_b1, wg_b2, wu_b2 = [], [], [], []
            for kt in range(KT):
                g = kpool.tile([P, 768], BF16, tag=f"wg_b1_{kt}")
                sync_dma(g[:], wg_d[kt * P:(kt + 1) * P, 256:1024], len(d_chain) - 3)
                wg_b1.append(g)
                u = kpool.tile([P, 768], BF16, tag=f"wu_b1_{kt}")
                sync_dma(u[:], wu_d[kt * P:(kt + 1) * P, 256:1024], len(d_chain) - 3)
                wu_b1.append(u)
            for kt in range(KT):
                g = kpool.tile([P, 1024], BF16, tag=f"wg_b2_{kt}")
                sync_dma(g[:], wg_d[kt * P:(kt + 1) * P, 1024:2048], len(d_chain) - 3)
                wg_b2.append(g)
                u = kpool.tile([P, 1024], BF16, tag=f"wu_b2_{kt}")
                sync_dma(u[:], wu_d[kt * P:(kt + 1) * P, 1024:2048], len(d_chain) - 3)
                wu_b2.append(u)

            wd_sb = wdpool.tile([P, IT, H], BF16)
            sync_dma(wd_sb[:, 0:8, :], wd_r[:, 0:8, :], len(d_chain) - 3)
            sync_dma(wd_sb[:, 8:16, :], wd_r[:, 8:16, :], len(d_chain) - 3)

            def gsl(it):
                if it == 0:
                    return lambda kt: wg_it0[:, kt, :]
                if it == 1:
                    return lambda kt: wg_it1[:, kt, :]
                if it < 8:
                    lo = (it - 2) * 128
                    return lambda kt: wg_b1[kt][:, lo:lo + 128]
                lo = (it - 8) * 128
                return lambda kt: wg_b2[kt][:, lo:lo + 128]

            def usl(it):
                if it == 0:
                    return lambda kt: wu_it0[:, kt, :]
                if it == 1:
                    return lambda kt: wu_it1[:, kt, :]
                if it < 8:
                    lo = (it - 2) * 128
                    return lambda kt: wu_b1[kt][:, lo:lo + 128]
                lo = (it - 8) * 128
                return lambda kt: wu_b2[kt][:, lo:lo + 128]

            hT_sb = hpool.tile([P, IT, T], BF16)

            for it in range(IT):
                g = gsl(it)
                u = usl(it)
                pg = psum.tile([P, T], F32, tag="pg")
                pu = psum.tile([P, T], F32, tag="pu")
                for kt in range(KT):
                    nc.tensor.matmul(
                        pg[:], g(kt), x_sb[:, kt, :],
                        start=(kt == 0), stop=(kt == KT - 1),
                    )
                for kt in range(KT):
                    nc.tensor.matmul(
                        pu[:], u(kt), x_sb[:, kt, :],
                        start=(kt == 0), stop=(kt == KT - 1),
                    )
                sg = spool.tile([P, T], F32)
                nc.scalar.activation(
                    sg[:], pg[:], mybir.ActivationFunctionType.Silu
                )
                nc.vector.tensor_mul(hT_sb[:, it, :], sg[:], pu[:])

            # phase 2: y[mt] = sum_it hT[:, it, mt-slice].T @ Wd[it]
            for mt in range(MT):
                ms = slice(mt * P, (mt + 1) * P)
                if mt < MT - 1:
                    py0 = psum.tile([P, 512], F32, tag="py0")
                    py1 = psum.tile([P, 512], F32, tag="py1")
                    for it in range(IT):
                        lhsT = hT_sb[:, it, ms]
                        nc.tensor.matmul(
                            py0[:], lhsT, wd_sb[:, it, 0:512],
                            start=(it == 0), stop=(it == IT - 1),
                        )
                        nc.tensor.matmul(
                            py1[:], lhsT, wd_sb[:, it, 512:1024],
                            start=(it == 0), stop=(it == IT - 1),
                        )
                    y0 = spool.tile([P, 512], F32, tag="y0")
                    nc.scalar.copy(y0[:], py0[:])
                    nc.sync.dma_start(y_d[ms, 0:512], y0[:])
                    y1 = spool.tile([P, 512], F32, tag="y1")
                    nc.vector.tensor_copy(y1[:], py1[:])
                    nc.sync.dma_start(y_d[ms, 512:1024], y1[:])
                else:
                    # last token tile: serial chains with shrinking width so
                    # each chunk's copy+DMA overlaps the next chain's matmuls
                    # and only a [128,128] copy+DMA trails the last matmul.
                    py0 = psum.tile([P, 512], F32, tag="py0")
                    for it in range(IT):
                        nc.tensor.matmul(
                            py0[:], hT_sb[:, it, ms], wd_sb[:, it, 0:512],
                            start=(it == 0), stop=(it == IT - 1),
                        )
                    y0 = spool.tile([P, 512], F32, tag="y0")
                    nc.scalar.copy(y0[:], py0[:])
                    nc.sync.dma_start(y_d[ms, 0:512], y0[:])
                    py1 = psum.tile([P, 256], F32, tag="py1", name="py1a")
                    for it in range(IT):
                        nc.tensor.matmul(
                            py1[:], hT_sb[:, it, ms], wd_sb[:, it, 512:768],
                            start=(it == 0), stop=(it == IT - 1),
                        )
                    y1 = spool.tile([P, 256], F32, tag="y1")
                    nc.scalar.copy(y1[:], py1[:])
                    nc.sync.dma_start(y_d[ms, 512:768], y1[:])
                    py2 = psum.tile([P, 128], F32, tag="pg", name="py2")
                    for it in range(IT):
                        nc.tensor.matmul(
                            py2[:], hT_sb[:, it, ms], wd_sb[:, it, 768:896],
                            start=(it == 0), stop=(it == IT - 1),
                        )
                    y2 = spool.tile([P, 128], F32, tag="y2")
                    nc.vector.tensor_copy(y2[:], py2[:])
                    nc.scalar.dma_start(y_d[ms, 768:896], y2[:])
                    py3 = psum.tile([P, 128], F32, tag="pu", name="py3")
                    for it in range(IT):
                        nc.tensor.matmul(
                            py3[:], hT_sb[:, it, ms], wd_sb[:, it, 896:1024],
                            start=(it == 0), stop=(it == IT - 1),
                        )
                    y3 = spool.tile([P, 128], F32, tag="y3")
                    nc.vector.tensor_copy(y3[:], py3[:])
                    nc.scalar.dma_start(y_d[ms, 896:1024], y3[:])

    nc.compile()
    return nc


def _get_compiled():
    global _compiled
    if _compiled is None:
        _compiled = _build()
    return _compiled


def _numpy_fallback(hidden_states, gate_kernel, up_kernel, down_kernel, group_sizes):
    # Exact reference math on host; only used for unexpected group_sizes.
    out = np.empty((hidden_states.shape[0], down_kernel.shape[2]), np.float32)
    start = 0
    for e in range(gate_kernel.shape[0]):
        g = int(group_sizes[e])
        x = hidden_states[start:start + g]
        gate = x @ gate_kernel[e]
        up = x @ up_kernel[e]
        sig = np.where(
            gate >= 0,
            1.0 / (1.0 + np.exp(-np.clip(gate, 0, None))),
            np.exp(np.clip(gate, None, 0))
            / (1.0 + np.exp(np.clip(gate, None, 0))),
        )
        h = gate * sig * up
        out[start:start + g] = h @ down_kernel[e]
        start += g
    out[start:] = 0.0
    return out


def _make_in_maps(hidden_states, gate_kernel, up_kernel, down_kernel):
    bf = ml_dtypes.bfloat16
    in_maps = []
    for e in range(E):
        x_e = hidden_states[e * T:(e + 1) * T]
        in_maps.append({
            "xT": np.ascontiguousarray(x_e.T).astype(bf),
            "wg": np.ascontiguousarray(gate_kernel[e]).astype(bf),
            "wu": np.ascontiguousarray(up_kernel[e]).astype(bf),
            "wd": np.ascontiguousarray(down_kernel[e]).astype(bf),
        })
    return in_maps


def profile_run(inputs, tmpdir=None):
    """Dev helper (not used by grading): run with NTFF tracing, return exec ns."""
    nc = _get_compiled()
    in_maps = _make_in_maps(
        np.asarray(inputs["hidden_states"], np.float32),
        np.asarray(inputs["gate_kernel"], np.float32),
        np.asarray(inputs["up_kernel"], np.float32),
        np.asarray(inputs["down_kernel"], np.float32),
    )
    res = run_bass_kernel_spmd(
        nc, in_maps, core_ids=list(range(E)), trace=True, tmpdir=tmpdir
    )
    return res.exec_time_ns


def kernel(hidden_states, gate_kernel, up_kernel, down_kernel, group_sizes):
    hidden_states = np.asarray(hidden_states, dtype=np.float32)
    gate_kernel = np.asarray(gate_kernel, dtype=np.float32)
    up_kernel = np.asarray(up_kernel, dtype=np.float32)
    down_kernel = np.asarray(down_kernel, dtype=np.float32)
    gs = np.asarray(group_sizes)

    if not (gs.shape == (E,) and np.all(gs == T)):
        return _numpy_fallback(
            hidden_states, gate_kernel, up_kernel, down_kernel, gs
        )

    nc = _get_compiled()
    in_maps = _make_in_maps(hidden_states, gate_kernel, up_kernel, down_kernel)
    res = run_bass_kernel_spmd(nc, in_maps, core_ids=list(range(E)))
    return np.concatenate([res.results[e]["y"] for e in range(E)], axis=0)


# revision 5
# speedup vs baseline: 1.0712x; 1.0712x over previous
"""Trainium2 Bass kernel for Llama4TextExperts-style grouped MoE FFN (SwiGLU).

Full-input contract: kernel(**inputs) takes the complete unsharded tensors and
returns the full [4096, 1024] output. Internally: expert-parallel across the 8
NeuronCores — core e gets expert e's three weight matrices and that expert's
512-token group (tokens arrive pre-sorted by expert with equal group sizes).
All routing / transposition is done host-side in numpy; no collectives needed.

Per-core device program (three GEMMs, ~6.4 GFLOP):
  phase 1: gate^T = Wg^T-stationary @ x^T, up^T likewise; SwiGLU fused on
           ACT (Silu) + DVE (mul) into h^T [I, T] bf16 resident in SBUF.
  phase 2: y = h @ Wd with h^T slices stationary, Wd streaming from its
           natural [I, H] DRAM layout; y lands untransposed in PSUM.

Startup schedule (the PE stream is already at the bf16 streaming roofline,
so the win is at the edges):
  - 8 warm-up matmuls on zeroed SBUF run while the first DMAs land, so the
    HAM clock gate releases (1.2->2.4 GHz) right as real work starts.
  - it0/it1 use narrow i-major weight tiles ([P, 8, 128], one DMA each) so
    the first gate chain only waits for ~0.5 MB; x streams concurrently on
    the scalar HWDGE queue while weights use the sync HWDGE queue.
  - it2..15 use k-major bulk tiles (contiguous 1.5-2 KB DMA descriptor rows
    straight from the [H, I] weight layout - much cheaper to issue than
    column-chunked loads). Each DMA is gated on an earlier completion so at
    most ~3 are in flight and arrival order tracks consumption order.
  - the final down-proj token tile is split into shrinking chains
    (512/256/128/128 cols) so the exposed copy+DMA after the very last
    matmul is only [128, 128], written out on the otherwise-idle scalar
    queue.
"""

import numpy as np
import ml_dtypes

import concourse.bass as bass
import concourse.mybir as mybir
import concourse.tile as tile
from concourse.tile import add_dep_helper
from concourse import bacc
from concourse.bass_utils import run_bass_kernel_spmd

# Problem shape (hardcoded per contract)
E = 8          # experts == cores
T = 512        # tokens per expert group
H = 1024       # hidden
I = 2048       # intermediate
P = 128        # partitions
KT = H // P    # 8  k-tiles over hidden
IT = I // P    # 16 i-tiles over intermediate
MT = T // P    # 4  token tiles

BF16 = mybir.dt.bfloat16
F32 = mybir.dt.float32

_compiled = None


def _build():
    nc = bacc.Bacc(None)
    xT_d = nc.declare_dram_parameter("xT", [H, T], BF16, isOutput=False)
    wg_d = nc.declare_dram_parameter("wg", [H, I], BF16, isOutput=False)
    wu_d = nc.declare_dram_parameter("wu", [H, I], BF16, isOutput=False)
    wd_d = nc.declare_dram_parameter("wd", [I, H], BF16, isOutput=False)
    y_d = nc.declare_dram_parameter("y", [T, H], F32, isOutput=True)

    xT_r = xT_d.rearrange("(ko p) t -> p ko t", p=P)     # [128, 8, 512]
    wg_r = wg_d.rearrange("(ko p) i -> p ko i", p=P)     # [128, 8, 2048]
    wu_r = wu_d.rearrange("(ko p) i -> p ko i", p=P)
    wd_r = wd_d.rearrange("(io p) h -> p io h", p=P)     # [128, 16, 1024]

    with tile.TileContext(nc) as tc:
        with (
            tc.tile_pool(name="xpool", bufs=1) as xpool,
            tc.tile_pool(name="kpool", bufs=1) as kpool,
            tc.tile_pool(name="wdpool", bufs=1) as wdpool,
            tc.tile_pool(name="hpool", bufs=1) as hpool,
            tc.tile_pool(name="spool", bufs=3) as spool,
            tc.tile_pool(name="psum", bufs=2, space="PSUM") as psum,
        ):
            # PE warm-up: the HAM clock gate holds the PE at 1.2GHz until it
            # has been ~3.4us busy. 8 cold N=512 matmuls on zeroed scratch
            # cover the DMA-wait window so the real stream starts at 2.4GHz.
            warm = xpool.tile([P, 512], BF16, tag="warm", name="warm")
            nc.gpsimd.memset(warm[:], 0.0)
            pwarm = psum.tile([P, 512], F32, tag="pg", name="pwarm")
            for _ in range(8):
                nc.tensor.matmul(
                    pwarm[:], warm[:, 0:128], warm[:], start=True, stop=True
                )

            # --- x: four [P, 2, T] chunks on the scalar HWDGE queue, fully
            # serialized so x0 (the first chunk any matmul needs) lands as
            # early as possible instead of round-robining with the rest.
            x_sb = xpool.tile([P, KT, T], BF16, tag="x", name="x_sb")
            dx = []
            for kq in range(4):
                call = nc.scalar.dma_start(
                    x_sb[:, 2 * kq:2 * kq + 2, :], xT_r[:, 2 * kq:2 * kq + 2, :]
                )
                if kq >= 1:
                    add_dep_helper(
                        call.ins, dx[kq - 1].ins, reason="x staging throttle"
                    )
                dx.append(call)

            # --- weights on the sync HWDGE queue, one gated chain ~2 deep so
            # arrival order tracks consumption order (the DGE round-robins
            # across everything outstanding, so an ungated pile would finish
            # all together).
            d_chain = []

            def sync_dma(dst, src, gate_idx):
                call = nc.sync.dma_start(dst, src)
                if gate_idx >= 0:
                    add_dep_helper(
                        call.ins, d_chain[gate_idx].ins,
                        reason="weight staging throttle",
                    )
                d_chain.append(call)
                return call

            # Head: it0/it1 i-major tiles, each loaded as two k-halves so the
            # partial-K startup chains below only wait for 128KB.
            wg_it0 = xpool.tile([P, KT, 128], BF16, tag="wg_it0", name="wg_it0")
            wu_it0 = xpool.tile([P, KT, 128], BF16, tag="wu_it0", name="wu_it0")
            wg_it1 = xpool.tile([P, KT, 128], BF16, tag="wg_it1", name="wg_it1")
            wu_it1 = xpool.tile([P, KT, 128], BF16, tag="wu_it1", name="wu_it1")
            sync_dma(wg_it0[:, 0:4, :], wg_r[:, 0:4, 0:128], -1)
            sync_dma(wu_it0[:, 0:4, :], wu_r[:, 0:4, 0:128], -1)
            sync_dma(wg_it1[:, 0:4, :], wg_r[:, 0:4, 128:256], 0)
            sync_dma(wu_it1[:, 0:4, :], wu_r[:, 0:4, 128:256], 1)
            sync_dma(wg_it0[:, 4:8, :], wg_r[:, 4:8, 0:128], 2)
            sync_dma(wu_it0[:, 4:8, :], wu_r[:, 4:8, 0:128], 3)
            sync_dma(wg_it1[:, 4:8, :], wg_r[:, 4:8, 128:256], 4)
            sync_dma(wu_it1[:, 4:8, :], wu_r[:, 4:8, 128:256], 5)

            # Bulk: [P, KT, cols] tiles in the same (ko p) i layout, loaded
            # as big I-split chunks (contiguous 768B-1KB descriptor rows) in
            # i-tile consumption order. b1 covers i-cols 256:1024 (it2..7),
            # b2 covers 1024:2048 (it8..15).
            wg_b1 = kpool.tile([P, KT, 768], BF16, tag="wg_b1")
            wu_b1 = kpool.tile([P, KT, 768], BF16, tag="wu_b1")
            wg_b2 = kpool.tile([P, KT, 1024], BF16, tag="wg_b2")
            wu_b2 = kpool.tile([P, KT, 1024], BF16, tag="wu_b2")
            n = len(d_chain)
            sync_dma(wg_b1[:, :, 0:384], wg_r[:, :, 256:640], n - 2)
            sync_dma(wu_b1[:, :, 0:384], wu_r[:, :, 256:640], n - 1)
            sync_dma(wg_b1[:, :, 384:768], wg_r[:, :, 640:1024], n)
            sync_dma(wu_b1[:, :, 384:768], wu_r[:, :, 640:1024], n + 1)
            sync_dma(wg_b2[:, :, 0:512], wg_r[:, :, 1024:1536], n + 2)
            sync_dma(wu_b2[:, :, 0:512], wu_r[:, :, 1024:1536], n + 3)
            sync_dma(wg_b2[:, :, 512:1024], wg_r[:, :, 1536:2048], n + 4)
            sync_dma(wu_b2[:, :, 512:1024], wu_r[:, :, 1536:2048], n + 5)

            wd_sb = wdpool.tile([P, IT, H], BF16)
            sync_dma(wd_sb[:, 0:8, :], wd_r[:, 0:8, :], n + 6)
            sync_dma(wd_sb[:, 8:16, :], wd_r[:, 8:16, :], n + 7)

            def gsl(it):
                if it == 0:
                    return lambda kt: wg_it0[:, kt, :]
                if it == 1:
                    return lambda kt: wg_it1[:, kt, :]
                if it < 8:
                    lo = (it - 2) * 128
                    return lambda kt: wg_b1[:, kt, lo:lo + 128]
                lo = (it - 8) * 128
                return lambda kt: wg_b2[:, kt, lo:lo + 128]

            def usl(it):
                if it == 0:
                    return lambda kt: wu_it0[:, kt, :]
                if it == 1:
                    return lambda kt: wu_it1[:, kt, :]
                if it < 8:
                    lo = (it - 2) * 128
                    return lambda kt: wu_b1[:, kt, lo:lo + 128]
                lo = (it - 8) * 128
                return lambda kt: wu_b2[:, kt, lo:lo + 128]

            hT_sb = hpool.tile([P, IT, T], BF16)

            def swiglu(it, pg, pu):
                sg = spool.tile([P, T], F32)
                nc.scalar.activation(
                    sg[:], pg[:], mybir.ActivationFunctionType.Silu
                )
                nc.vector.tensor_mul(hT_sb[:, it, :], sg[:], pu[:])

            # it0/it1 run as partial-K chains: kt0-3 as soon as x0/x1 land,
            # kt4-7 once x2/x3 arrive. Keeps the PE fed while x streams in.
            pg01 = [
                psum.tile([P, T], F32, tag="pg", name=f"pg0{i}")
                for i in range(2)
            ]
            pu01 = [
                psum.tile([P, T], F32, tag="pu", name=f"pu0{i}")
                for i in range(2)
            ]
            for it in (0, 1):
                for kt in range(4):
                    nc.tensor.matmul(
                        pg01[it][:], gsl(it)(kt), x_sb[:, kt, :],
                        start=(kt == 0), stop=False,
                    )
                for kt in range(4):
                    nc.tensor.matmul(
                        pu01[it][:], usl(it)(kt), x_sb[:, kt, :],
                        start=(kt == 0), stop=False,
                    )
            for it in (0, 1):
                for kt in range(4, KT):
                    nc.tensor.matmul(
                        pg01[it][:], gsl(it)(kt), x_sb[:, kt, :],
                        start=False, stop=(kt == KT - 1),
                    )
                for kt in range(4, KT):
                    nc.tensor.matmul(
                        pu01[it][:], usl(it)(kt), x_sb[:, kt, :],
                        start=False, stop=(kt == KT - 1),
                    )
                swiglu(it, pg01[it], pu01[it])

            for it in range(2, IT):
                g = gsl(it)
                u = usl(it)
                pg = psum.tile([P, T], F32, tag="pg")
                pu = psum.tile([P, T], F32, tag="pu")
                for kt in range(KT):
                    nc.tensor.matmul(
                        pg[:], g(kt), x_sb[:, kt, :],
                        start=(kt == 0), stop=(kt == KT - 1),
                    )
                for kt in range(KT):
                    nc.tensor.matmul(
                        pu[:], u(kt), x_sb[:, kt, :],
                        start=(kt == 0), stop=(kt == KT - 1),
                    )
                swiglu(it, pg, pu)

            # phase 2: y[mt] = sum_it hT[:, it, mt-slice].T @ Wd[it]
            for mt in range(MT):
                ms = slice(mt * P, (mt + 1) * P)
                if mt < MT - 1:
                    py0 = psum.tile([P, 512], F32, tag="py0")
                    py1 = psum.tile([P, 512], F32, tag="py1")
                    for it in range(IT):
                        lhsT = hT_sb[:, it, ms]
                        nc.tensor.matmul(
                            py0[:], lhsT, wd_sb[:, it, 0:512],
                            start=(it == 0), stop=(it == IT - 1),
                        )
                        nc.tensor.matmul(
                            py1[:], lhsT, wd_sb[:, it, 512:1024],
                            start=(it == 0), stop=(it == IT - 1),
                        )
                    y0 = spool.tile([P, 512], F32, tag="y0")
                    nc.scalar.copy(y0[:], py0[:])
                    nc.sync.dma_start(y_d[ms, 0:512], y0[:])
                    y1 = spool.tile([P, 512], F32, tag="y1")
                    nc.vector.tensor_copy(y1[:], py1[:])
                    nc.sync.dma_start(y_d[ms, 512:1024], y1[:])
                else:
                    # last token tile: serial chains with shrinking width so
                    # each chunk's copy+DMA overlaps the next chain's matmuls
                    # and only a [128,128] copy+DMA trails the last matmul.
                    py0 = psum.tile([P, 512], F32, tag="py0")
                    for it in range(IT):
                        nc.tensor.matmul(
                            py0[:], hT_sb[:, it, ms], wd_sb[:, it, 0:512],
                            start=(it == 0), stop=(it == IT - 1),
                        )
                    y0 = spool.tile([P, 512], F32, tag="y0")
                    nc.scalar.copy(y0[:], py0[:])
                    nc.sync.dma_start(y_d[ms, 0:512], y0[:])
                    py1 = psum.tile([P, 256], F32, tag="py1", name="py1a")
                    for it in range(IT):
                        nc.tensor.matmul(
                            py1[:], hT_sb[:, it, ms], wd_sb[:, it, 512:768],
                            start=(it == 0), stop=(it == IT - 1),
                        )
                    y1 = spool.tile([P, 256], F32, tag="y1")
                    nc.scalar.copy(y1[:], py1[:])
                    nc.sync.dma_start(y_d[ms, 512:768], y1[:])
                    py2 = psum.tile([P, 128], F32, tag="pg", name="py2")
                    for it in range(IT):
                        nc.tensor.matmul(
                            py2[:], hT_sb[:, it, ms], wd_sb[:, it, 768:896],
                            start=(it == 0), stop=(it == IT - 1),
                        )
                    y2 = spool.tile([P, 128], F32, tag="y2")
                    nc.vector.tensor_copy(y2[:], py2[:])
                    nc.scalar.dma_start(y_d[ms, 768:896], y2[:])
                    py3 = psum.tile([P, 128], F32, tag="pu", name="py3")
                    for it in range(IT):
                        nc.tensor.matmul(
                            py3[:], hT_sb[:, it, ms], wd_sb[:, it, 896:1024],
                            start=(it == 0), stop=(it == IT - 1),
                        )
                    y3 = spool.tile([P, 128], F32, tag="y3")
                    nc.vector.tensor_copy(y3[:], py3[:])
                    nc.scalar.dma_start(y_d[ms, 896:1024], y3[:])

    nc.compile()
    return nc


def _get_compiled():
    global _compiled
    if _compiled is None:
        _compiled = _build()
    return _compiled


def _numpy_fallback(hidden_states, gate_kernel, up_kernel, down_kernel, group_sizes):
    # Exact reference math on host; only used for unexpected group_sizes.
    out = np.empty((hidden_states.shape[0], down_kernel.shape[2]), np.float32)
    start = 0
    for e in range(gate_kernel.shape[0]):
        g = int(group_sizes[e])
        x = hidden_states[start:start + g]
        gate = x @ gate_kernel[e]
        up = x @ up_kernel[e]
        sig = np.where(
            gate >= 0,
            1.0 / (1.0 + np.exp(-np.clip(gate, 0, None))),
            np.exp(np.clip(gate, None, 0))
            / (1.0 + np.exp(np.clip(gate, None, 0))),
        )
        h = gate * sig * up
        out[start:start + g] = h @ down_kernel[e]
        start += g
    out[start:] = 0.0
    return out


def _make_in_maps(hidden_states, gate_kernel, up_kernel, down_kernel):
    bf = ml_dtypes.bfloat16
    in_maps = []
    for e in range(E):
        x_e = hidden_states[e * T:(e + 1) * T]
        in_maps.append({
            "xT": np.ascontiguousarray(x_e.T).astype(bf),
            "wg": np.ascontiguousarray(gate_kernel[e]).astype(bf),
            "wu": np.ascontiguousarray(up_kernel[e]).astype(bf),
            "wd": np.ascontiguousarray(down_kernel[e]).astype(bf),
        })
    return in_maps


def profile_run(inputs, tmpdir=None):
    """Dev helper (not used by grading): run with NTFF tracing, return exec ns."""
    nc = _get_compiled()
    in_maps = _make_in_maps(
        np.asarray(inputs["hidden_states"], np.float32),
        np.asarray(inputs["gate_kernel"], np.float32),
        np.asarray(inputs["up_kernel"], np.float32),
        np.asarray(inputs["down_kernel"], np.float32),
    )
    res = run_bass_kernel_spmd(
        nc, in_maps, core_ids=list(range(E)), trace=True, tmpdir=tmpdir
    )
    return res.exec_time_ns


def kernel(hidden_states, gate_kernel, up_kernel, down_kernel, group_sizes):
    hidden_states = np.asarray(hidden_states, dtype=np.float32)
    gate_kernel = np.asarray(gate_kernel, dtype=np.float32)
    up_kernel = np.asarray(up_kernel, dtype=np.float32)
    down_kernel = np.asarray(down_kernel, dtype=np.float32)
    gs = np.asarray(group_sizes)

    if not (gs.shape == (E,) and np.all(gs == T)):
        return _numpy_fallback(
            hidden_states, gate_kernel, up_kernel, down_kernel, gs
        )

    nc = _get_compiled()
    in_maps = _make_in_maps(hidden_states, gate_kernel, up_kernel, down_kernel)
    res = run_bass_kernel_spmd(nc, in_maps, core_ids=list(range(E)))
    return np.concatenate([res.results[e]["y"] for e in range(E)], axis=0)
